# revision 16
# baseline (speedup 1.0000x reference)
"""Multi-head attention Trainium2 kernel (v4: row-paired scores + fp8-DR ctx).

Problem: B=4, S=2048, D_MODEL=1024, H=16 heads, d_k=d_v=64.

Sharding (8 cores, no collectives): core c handles batch b=c//2 and head
group g=c%2 (8 heads). Each core computes its 8 heads' attention and the
partial output projection ctx @ Wo[g's rows]; the host sums the two
head-group partials per batch and adds the (folded) biases.

v4 structure (measured on this rig via mb.py):
 - K=64 matmuls issued back-to-back on row tiles (0,0)/(64,0) run
   CONCURRENTLY (~220ns/pair at N=512 vs 249ns for one K=128 matmul).
   The scores matmuls (dk=64, lhsT partitions 0-63/64-127) are emitted as
   adjacent pairs to exploit this: scores cost ~halves.
 - fp8e4 DoubleRow matmuls contract K=256 (two 128-k-tiles, 3D AP
   [Ki, Ko=2, dim]) in ~246ns = 2x bf16 throughput. ctx uses this: e and
   vha stored fp8e4; per (head, qc) the s=2048 contraction is 8 DR matmuls.
   The [vh | ones] 65th column still yields the softmax denominator Z.
   (host sim: rel_err 1.5e-2 vs 2e-2 budget; all-bf16 fallback CTX_MODE=
   colpair uses M=64 col-tile pairs + a separate ones-lhsT Z chain.)
 - Fine-grained pipeline per (head-pair p, q-chunk qc): for each s-tile:
   2 paired score matmuls -> one [128,1024] exp on ScalarE -> ctx DR after
   each odd s-tile. ScalarE (256 exps, ~1.1us each) is the pacing engine;
   K/Q/V/out projections are emitted as fillers inside the sweep to soak
   the PE slack under the exp pacing.
 - PSUM: score tiles [128,2,512] (2 banks) bufs=2; ctx accumulators 2
   banks; proj/outproj 2 banks = 8 total.
"""

import os

import numpy as np
import ml_dtypes

import concourse.bass as bass
import concourse.bacc as bacc
import concourse.mybir as mybir
import concourse.tile as tile
from concourse.bass import ts

BF16 = mybir.dt.bfloat16
F8 = mybir.dt.float8e4
F32 = mybir.dt.float32

RECIP_MODE = os.environ.get("RECIP_MODE", "approx")  # approx | exact
CTX_MODE = os.environ.get("CTX_MODE", "dr")  # dr | colpair
EXP_COLS = int(os.environ.get("EXP_COLS", "0"))  # >0: tiny exp (timing diag)
NO_CTX = os.environ.get("NO_CTX", "0") == "1"  # skip ctx matmuls (timing diag)

D_MODEL, D_K, D_V, N_HEADS = 1024, 64, 64, 16
B, S = 4, 2048
N_CORES = 8
NH = 8            # heads per core
HD = NH * D_V     # 512
T = S
DC = 8            # D_MODEL / 128
TCN = 4           # t chunks of 512
SCN = 16          # s tiles of 128
QCN = 4           # q chunks of 512
HCN = 4           # hd chunks of 128 (one head pair each)
VPAD = 72         # padded per-head vha row (65 used) for 16B-aligned DR strides


def build_nc(reps: int = 1, phases: str = "all"):
    nc = bacc.Bacc("TRN2", target_bir_lowering=False, debug=False)

    xq_d = nc.dram_tensor("xq_t", [128, DC, T], BF16, kind="ExternalInput")
    xk_d = nc.dram_tensor("xk_t", [128, DC, T], BF16, kind="ExternalInput")
    xv_d = nc.dram_tensor("xv_t", [128, DC, T], BF16, kind="ExternalInput")
    wq_d = nc.dram_tensor("wq", [128, DC, HD], BF16, kind="ExternalInput")
    wk_d = nc.dram_tensor("wk", [128, DC, HD], BF16, kind="ExternalInput")
    wv_d = nc.dram_tensor("wv", [128, DC, HD], BF16, kind="ExternalInput")
    wo_d = nc.dram_tensor("wo", [128, HCN, D_MODEL], BF16, kind="ExternalInput")
    bq_d = nc.dram_tensor("bq", [128, HCN], F32, kind="ExternalInput")
    out_d = nc.dram_tensor("out", [SCN, 128, D_MODEL], F32, kind="ExternalOutput")

    with tile.TileContext(nc) as tc:
        def body():
            emit_body(nc, tc, xq_d, xk_d, xv_d, wq_d, wk_d, wv_d, wo_d, bq_d, out_d, phases)

        if reps == 1:
            body()
        else:
            with tc.For_i(0, reps, 1):
                body()
    nc.compile()
    return nc


def emit_body(nc, tc, xq_d, xk_d, xv_d, wq_d, wk_d, wv_d, wo_d, bq_d, out_d, phases="all"):
    import contextlib

    dr = CTX_MODE == "dr"
    e_dt = F8 if dr else BF16
    v_dt = F8 if dr else BF16

    ctx = contextlib.ExitStack()
    with ctx:
        wpool = ctx.enter_context(tc.tile_pool(name="wpool", bufs=1))
        qkpool = ctx.enter_context(tc.tile_pool(name="qkpool", bufs=1))
        vpool = ctx.enter_context(tc.tile_pool(name="vpool", bufs=1))
        cpool = ctx.enter_context(tc.tile_pool(name="cpool", bufs=1))
        xst = ctx.enter_context(tc.tile_pool(name="xst", bufs=6))
        vst = ctx.enter_context(tc.tile_pool(name="vst", bufs=6))
        expool = ctx.enter_context(tc.tile_pool(name="expool", bufs=6))
        zpool = ctx.enter_context(tc.tile_pool(name="zpool", bufs=2))
        opool = ctx.enter_context(tc.tile_pool(name="opool", bufs=2))
        ps = ctx.enter_context(tc.tile_pool(name="ps", bufs=2, space="PSUM"))
        ctp = ctx.enter_context(tc.tile_pool(name="ctp", bufs=1, space="PSUM"))
        cp = ctx.enter_context(tc.tile_pool(name="cp", bufs=2, space="PSUM"))

        wq_sb = wpool.tile([128, DC, HD], BF16, tag="wq")
        wk_sb = wpool.tile([128, DC, HD], BF16, tag="wk")
        wv_sb = wpool.tile([128, DC, HD], BF16, tag="wv")
        wo_sb = wpool.tile([128, HCN, D_MODEL], BF16, tag="wo")
        bq_sb = wpool.tile([128, HCN], F32, tag="bq")

        nc.sync.dma_start(wk_sb[:], wk_d[:])
        nc.sync.dma_start(wq_sb[:], wq_d[:])
        nc.sync.dma_start(wv_sb[:], wv_d[:])
        nc.sync.dma_start(wo_sb[:], wo_d[:])
        nc.sync.dma_start(bq_sb[:], bq_d[:])

        qhT = qkpool.tile([128, HCN, T], BF16, tag="qhT")  # [hd%128, hc, t]
        khT = qkpool.tile([128, HCN, T], BF16, tag="khT")
        # [s%128, s//128, h, dv | ones | pad]
        vha = vpool.tile([128, SCN, NH, VPAD], v_dt, tag="vha")
        ctxT = cpool.tile([128, HCN, T], BF16, tag="ctxT")
        if dr:
            nc.vector.memset(vha[:, :, :, D_V : D_V + 1], 1.0)
        else:
            ones = vpool.tile([128, 2], BF16, tag="ones")
            nc.vector.memset(ones[:], 1.0)
        if NO_CTX:
            nc.vector.memset(ctxT[:], 0.5)

        def kproj_hc(hc, tc_i, xt):
            pt = cp.tile([128, 512], F32, tag="u", name="kp_t")
            for dc in range(DC):
                nc.tensor.matmul(
                    pt[:],
                    lhsT=wk_sb[:, dc, ts(hc, 128)],
                    rhs=xt[:, dc, :],
                    start=(dc == 0),
                    stop=(dc == DC - 1),
                )
            nc.vector.tensor_copy(khT[:, hc, ts(tc_i, 512)], pt[:])

        def qproj_hc(hc, tc_i, xt):
            pt = cp.tile([128, 512], F32, tag="u", name="qp_t")
            for dc in range(DC):
                nc.tensor.matmul(
                    pt[:],
                    lhsT=wq_sb[:, dc, ts(hc, 128)],
                    rhs=xt[:, dc, :],
                    start=(dc == 0),
                    stop=(dc == DC - 1),
                )
            nc.vector.tensor_scalar_add(
                qhT[:, hc, ts(tc_i, 512)], pt[:], bq_sb[:, hc : hc + 1]
            )

        def load_xk(tc_i):
            xt = xst.tile([128, DC, 512], BF16, tag="x", name="xk_t")
            nc.sync.dma_start(xt[:], xk_d[:, :, ts(tc_i, 512)])
            return xt

        def load_xq(tc_i):
            xt = xst.tile([128, DC, 512], BF16, tag="x", name="xq_t")
            nc.sync.dma_start(xt[:], xq_d[:, :, ts(tc_i, 512)])
            return xt

        def vproj(sc):
            xt = vst.tile([128, DC, 128], BF16, tag="xv", name="xv_t")
            nc.sync.dma_start(xt[:], xv_d[:, :, ts(sc, 128)])
            pv = cp.tile([128, 512], F32, tag="u", name="pv_t")
            for dc in range(DC):
                nc.tensor.matmul(
                    pv[:],
                    lhsT=xt[:, dc, :],
                    rhs=wv_sb[:, dc, :],
                    start=(dc == 0),
                    stop=(dc == DC - 1),
                )
            nc.vector.tensor_copy(
                vha[:, sc, :, 0:D_V], pv[:].rearrange("p (h d) -> p h d", d=D_V)
            )

        # ---------------- attention unit machinery ----------------
        # state per (p, qc): e-pair tiles + ctx accumulators
        def scores_sc(p, qc, sc, st, ep):
            """2 row-paired score matmuls for s-tile sc + the exp."""
            jj = sc % 2
            for hl in range(2):
                pb = hl * 64
                nc.tensor.matmul(
                    st[:, hl, :],
                    lhsT=khT[pb : pb + 64, p, ts(sc, 128)],
                    rhs=qhT[pb : pb + 64, p, ts(qc, 512)],
                    start=True,
                    stop=True,
                    tile_position=(pb, 0),
                )
            if EXP_COLS:
                nc.scalar.activation(
                    ep[:, jj, :, 0:EXP_COLS],
                    st[:, :, 0:EXP_COLS],
                    mybir.ActivationFunctionType.Exp,
                    scale=0.125,
                )
            else:
                nc.scalar.activation(
                    ep[:, jj, :, :],
                    st[:],
                    mybir.ActivationFunctionType.Exp,
                    scale=0.125,
                )

        def ctx_pair_dr(p, qc, j, ep, ctA, ctB, first, last):
            for hl in range(2):
                h = 2 * p + hl
                ct = ctA if hl == 0 else ctB
                nc.tensor.matmul(
                    ct[0 : D_V + 1, :],
                    lhsT=vha[:, 2 * j : 2 * j + 2, h, 0 : D_V + 1],
                    rhs=ep[:, :, hl, :],
                    start=first,
                    stop=last,
                    perf_mode=mybir.MatmulPerfMode.DoubleRow,
                    skip_group_check=True,
                )

        def ctx_pair_colpair(p, qc, j, ep, ctA, zt, first, last):
            for jj in range(2):
                sc = 2 * j + jj
                f = first and jj == 0
                l = last and jj == 1
                for hl in range(2):
                    h = 2 * p + hl
                    nc.tensor.matmul(
                        ctA[hl * 64 : hl * 64 + 64, :],
                        lhsT=vha[:, sc, h, 0:D_V],
                        rhs=ep[:, jj, hl, :],
                        start=f and hl == 0,
                        stop=l,
                        tile_position=(0, hl * 64),
                        skip_group_check=True,
                    )
                for hl in range(2):
                    nc.tensor.matmul(
                        zt[hl * 32 : hl * 32 + 1, :],
                        lhsT=ones[:, hl : hl + 1],
                        rhs=ep[:, jj, hl, :],
                        start=f and hl == 0,
                        stop=l,
                        tile_position=(0, hl * 32),
                        skip_group_check=True,
                    )

        def norm_head(p, qc, hl, ct, zsrc):
            pb = hl * 64
            tz = zpool.tile([1, 512], F32, tag="tz", name="tz_t")
            nc.vector.tensor_copy(tz[:], zsrc)
            rz = zpool.tile([1, 512], F32, tag="rz", name="rz_t")
            if RECIP_MODE == "exact":
                nc.vector.reciprocal(rz[:], tz[:])
            else:
                nc.vector.reciprocal_approx_fast(rz[:], tz[:])
            bc = zpool.tile([64, 512], F32, tag="bc", name="bc_t")
            nc.gpsimd.partition_broadcast(bc[:], rz[:], channels=64)
            nc.vector.tensor_mul(
                ctxT[pb : pb + 64, p, ts(qc, 512)], ct, bc[:]
            )

        def outproj_d2(qt, d2):
            po = cp.tile([128, 512], F32, tag="u", name="po_t")
            for hc in range(HCN):
                nc.tensor.matmul(
                    po[:],
                    lhsT=ctxT[:, hc, ts(qt, 128)],
                    rhs=wo_sb[:, hc, ts(d2, 512)],
                    start=(hc == 0),
                    stop=(hc == HCN - 1),
                )
            o_sb = opool.tile([128, 512], F32, tag="o", name="o_sb")
            nc.vector.tensor_copy(o_sb[:], po[:])
            nc.sync.dma_start(out_d[qt, :, ts(d2, 512)], o_sb[:])

        def attn_unit(p, qc, slots):
            """Full scores+exp+ctx+norm for one (head-pair, q-chunk).

            slots: 4 lists of 0-arg filler callables, one per 4-s-tile
            block. Within a block the PE stream is [4x paired scores
            (row-tile mode)] [fillers (full mode)] [4x ctx (DR mode)] to
            keep tiling-mode switches to ~3 per block (~100ns each).
            """
            if dr:
                ctA = ctp.tile([128, 512], F32, tag="ctA", name="ctA")
                ctB = ctp.tile([128, 512], F32, tag="ctB", name="ctB")
                zt = None
            else:
                ctA = ctp.tile([128, 512], F32, tag="ctA", name="ctA")
                zt = ctp.tile([128, 512], F32, tag="ctB", name="zt")
            eps = {}
            for blk in range(4):
                for sc in range(4 * blk, 4 * blk + 4):
                    j = sc // 2
                    if sc % 2 == 0:
                        eps[j] = expool.tile(
                            [128, 2, 2, 512], e_dt, tag="exp", name="exp_t"
                        )
                    st = ps.tile([128, 2, 512], F32, tag="s", name="s_ps")
                    scores_sc(p, qc, sc, st, eps[j])
                for f in slots[blk]:
                    f()
                if not NO_CTX:
                    for j in (2 * blk, 2 * blk + 1):
                        ep = eps.pop(j)
                        if dr:
                            ctx_pair_dr(p, qc, j, ep, ctA, ctB, j == 0, j == SCN // 2 - 1)
                        else:
                            ctx_pair_colpair(p, qc, j, ep, ctA, zt, j == 0, j == SCN // 2 - 1)
            if NO_CTX:
                return
            if dr:
                norm_head(p, qc, 0, ctA[0:D_V, :], ctA[D_V : D_V + 1, :])
                norm_head(p, qc, 1, ctB[0:D_V, :], ctB[D_V : D_V + 1, :])
            else:
                norm_head(p, qc, 0, ctA[0:D_V, :], zt[0:1, :])
                norm_head(p, qc, 1, ctA[D_V : 2 * D_V, :], zt[32:33, :])

        # ------------------- emission schedule -------------------
        # x-chunk tiles are shared across all 4 head-chunks of a t-chunk;
        # the cache keeps the python reference so later kproj_hc/qproj_hc
        # fillers reuse the same SBUF tile (pool WAR handles rotation, and
        # filler ordering keeps each chunk's consumers within ~2 allocs).
        xcache = {}

        def mk_k(hc, tc_i):
            def f():
                key = ("k", tc_i)
                if key not in xcache:
                    xcache[key] = load_xk(tc_i)
                kproj_hc(hc, tc_i, xcache[key])
            return f

        def mk_q(hc, qc_i):
            def f():
                key = ("q", qc_i)
                if key not in xcache:
                    xcache[key] = load_xq(qc_i)
                qproj_hc(hc, qc_i, xcache[key])
            return f

        def mk_v(sc):
            return lambda: vproj(sc)

        def mk_o(qt, d2):
            return lambda: outproj_d2(qt, d2)

        outw = {
            qc: [mk_o(qt, d2) for qt in range(4 * qc, 4 * qc + 4) for d2 in range(2)]
            for qc in range(QCN)
        }

        # Prologue: kh hc=0 t-chunk 0 + qh hc=0 qc=0 -> first scores fire
        # ~4us in; all of vproj and the rest of kproj ride unit (0,0)'s
        # filler slots (each slot sits between a block's scores and its ctx,
        # so vproj(s-tiles of block b) lands just before the ctx that needs
        # them).
        mk_k(0, 0)()
        mk_q(0, 0)()

        attn_unit(0, 0, [
            [mk_v(0), mk_v(1), mk_v(2), mk_v(3), mk_k(0, 1)],
            [mk_v(4), mk_v(5), mk_v(6), mk_v(7), mk_k(0, 2)],
            [mk_v(8), mk_v(9), mk_v(10), mk_v(11), mk_k(0, 3)],
            [mk_v(12), mk_v(13), mk_v(14), mk_v(15)],
        ])
        for p in range(1, HCN):
            mk_k(p, 0)()
            mk_q(p, 0)()
            slots = [[mk_k(p, 1)], [mk_k(p, 2)], [mk_k(p, 3)], []]
            if p == 3:
                slots[3] = [mk_q(hc, 1) for hc in range(HCN)]
            attn_unit(p, 0, slots)

        for qc in range(1, QCN):
            for p in range(HCN):
                o = outw[qc - 1]
                if p == 0:
                    slots = [[o[0]], [o[1]], [o[2]], [o[3]]]
                elif p == 1:
                    slots = [[o[4]], [o[5]], [o[6]], [o[7]]]
                elif p == 2:
                    slots = [[mk_q(0, qc + 1)], [mk_q(1, qc + 1)], [], []] \
                        if qc < QCN - 1 else [[], [], [], []]
                else:
                    slots = [[mk_q(2, qc + 1)], [mk_q(3, qc + 1)], [], []] \
                        if qc < QCN - 1 else [[], [], [], []]
                attn_unit(p, qc, slots)
        for f in outw[QCN - 1]:
            f()


# ---------------------------------------------------------------------------
# host side
# ---------------------------------------------------------------------------

_NC_CACHE = {}


def _get_nc(reps: int = 1):
    if reps not in _NC_CACHE:
        _NC_CACHE[reps] = build_nc(reps)
    return _NC_CACHE[reps]


def _to_bf16(a):
    return np.ascontiguousarray(a).astype(ml_dtypes.bfloat16)


def make_in_maps(q, k, v, Wq, bq, Wk, bk, Wv, bv, Wo, bo):
    """Build the per-core input maps (host-side sharding + layout)."""
    in_maps = []
    for c in range(N_CORES):
        b = c // 2
        hg = c % 2
        hs = slice(hg * NH, hg * NH + NH)

        def xt(x):
            # (S, D) -> [p, dc, t] bf16 with D = dc*128 + p
            return _to_bf16(
                np.asarray(x, np.float32).T.reshape(DC, 128, T).transpose(1, 0, 2)
            )

        def wproj(W):
            # (8, 1024, 64) -> [p, dc, hd]  (hd = h*64+dv, D = dc*128+p)
            Wc = np.asarray(W[hs], np.float32).transpose(1, 0, 2).reshape(D_MODEL, HD)
            return _to_bf16(Wc.reshape(DC, 128, HD).transpose(1, 0, 2))

        wo_c = np.asarray(Wo[hg * HD : (hg + 1) * HD], np.float32)  # (512, 1024)
        bq_c = np.asarray(bq[hs], np.float32).reshape(HD)  # (512,)

        in_maps.append(
            {
                "xq_t": xt(q[b]),
                "xk_t": xt(k[b]),
                "xv_t": xt(v[b]),
                "wq": wproj(Wq),
                "wk": wproj(Wk),
                "wv": wproj(Wv),
                "wo": _to_bf16(wo_c.reshape(HCN, 128, D_MODEL).transpose(1, 0, 2)),
                "bq": np.ascontiguousarray(bq_c.reshape(HCN, 128).T),
            }
        )
    return in_maps


def combine_outputs(results, bv, Wo, bo):
    """results: list of 8 dicts with 'out' (16,128,1024). Returns (B,S,D)."""
    bo_eff = np.asarray(bo, np.float32) + np.asarray(bv, np.float32).reshape(-1) @ np.asarray(
        Wo, np.float32
    )
    out = np.empty((B, S, D_MODEL), np.float32)
    for b in range(B):
        p0 = results[2 * b]["out"].reshape(S, D_MODEL)
        p1 = results[2 * b + 1]["out"].reshape(S, D_MODEL)
        out[b] = p0 + p1 + bo_eff
    return out


def kernel(q, k, v, Wq, bq, Wk, bk, Wv, bv, Wo, bo):
    from concourse.bass_utils import run_bass_kernel_spmd

    nc = _get_nc(1)
    in_maps = make_in_maps(q, k, v, Wq, bq, Wk, bk, Wv, bv, Wo, bo)
    res = run_bass_kernel_spmd(nc, in_maps, core_ids=list(range(N_CORES)))
    return combine_outputs(res.results, bv, Wo, bo)
